# revision 22
# baseline (speedup 1.0000x reference)
"""Trainium2 Bass kernel for nn_AutoRegerting_2954937500106.

Self-contained: builds an 8-core SPMD Bass program and reassembles the
full [B, T, V] output.

Strategy (v3):
  - Recurrence: 8-way tensor-parallel over the gate dim, LayerNorm folded
    into the gate matmuls (raw-h slices + bn_stats partials shipped in the
    AllGather; consumer corrects with rstd/mean rank-1 terms).  Gates in
    batch-major [16,384] layout; rstd via one batched Newton-rsqrt chain
    on gpsimd; two AllGathers per step (one per layer) to pipeline the
    layer loops.
  - Head (W1 -> LeakyReLU -> LN2 -> W2) in fp16, V-sharded, with BOTH
    LayerNorms folded into the adjacent matmuls via extra contraction
    rows; LN2's rstd applied as a per-partition scale on the output copy.
    Head work is interleaved into the recurrence loop (pulled as small op
    groups) to keep the PE warm through the AllGather waits and eliminate
    the tail.
  - gi0 precomputed batched over T in fp32 (feeds the chaotic recurrence).
"""
import sys as _sys
for _p in ("/opt/trn_rl_repo", "/opt/trn_rl_repo/concourse"):
    if _p not in _sys.path:
        _sys.path.append(_p)

import numpy as np
import concourse.bacc as bacc
import concourse.bass as bass
import concourse.mybir as mybir
import concourse.tile as tile

F32 = mybir.dt.float32
F16 = mybir.dt.float16
I32 = mybir.dt.int32
AF = mybir.ActivationFunctionType
ALU = mybir.AluOpType

H = 1024
E = 512
B = 16
V = 32000
T = 256
BT = T * B
NCORES = 8
KH = H // 128
KE = E // 128
MSL = 3 * 128
VC = V // NCORES
EPS = 1e-5
NEG_SLOPE = 0.01
QK = 0x5F3759DF
PAY = 128 * 16 + 16 * 6
NB = 8           # head batches (32 steps x 16 = 512 cols each)
HBS = T // NB    # 32 steps per head batch
INTERLEAVE_HEAD = True


def build_nc(n_cores=NCORES):
    nc = bacc.Bacc("TRN2", target_bir_lowering=False, debug=False,
                   enable_asserts=False, num_devices=n_cores)

    xT    = nc.dram_tensor("xT",    [KE, 128, BT], F32, kind="ExternalInput").ap()
    wih0  = nc.dram_tensor("wih0",  [E, MSL], F32, kind="ExternalInput").ap()
    whh0  = nc.dram_tensor("whh0",  [H, MSL], F32, kind="ExternalInput").ap()
    wih1  = nc.dram_tensor("wih1",  [H, MSL], F32, kind="ExternalInput").ap()
    whh1  = nc.dram_tensor("whh1",  [H, MSL], F32, kind="ExternalInput").ap()
    bc0   = nc.dram_tensor("bc0",   [128, MSL], F32, kind="ExternalInput").ap()
    cin0  = nc.dram_tensor("cin0",  [16, MSL], F32, kind="ExternalInput").ap()
    cinI  = nc.dram_tensor("cinI",  [16, MSL], F32, kind="ExternalInput").ap()
    cinH  = nc.dram_tensor("cinH",  [16, MSL], F32, kind="ExternalInput").ap()
    vw0b  = nc.dram_tensor("vw0b",  [16, MSL], F32, kind="ExternalInput").ap()
    vwib  = nc.dram_tensor("vwib",  [16, MSL], F32, kind="ExternalInput").ap()
    vwhb  = nc.dram_tensor("vwhb",  [16, MSL], F32, kind="ExternalInput").ap()
    wT0b  = nc.dram_tensor("wT0b",  [16, 128], F32, kind="ExternalInput").ap()
    wT1b  = nc.dram_tensor("wT1b",  [16, 128], F32, kind="ExternalInput").ap()
    lnbT0 = nc.dram_tensor("lnbT0", [16, 128], F32, kind="ExternalInput").ap()
    lnbT1 = nc.dram_tensor("lnbT1", [16, 128], F32, kind="ExternalInput").ap()
    eye16 = nc.dram_tensor("eye16", [16, 16], F32, kind="ExternalInput").ap()
    w1sb  = nc.dram_tensor("w1sb",  [H, H], F16, kind="ExternalInput").ap()
    w1xb  = nc.dram_tensor("w1xb",  [2, H], F16, kind="ExternalInput").ap()
    w2sb  = nc.dram_tensor("w2sb",  [H, VC], F16, kind="ExternalInput").ap()
    w2xb  = nc.dram_tensor("w2xb",  [2, VC], F16, kind="ExternalInput").ap()
    out   = nc.dram_tensor("out",   [BT, VC], F32, kind="ExternalOutput").ap()

    rg = [list(range(n_cores))]

    with tile.TileContext(nc) as tc:
        with tc.tile_pool(name="dramp", bufs=1, space="DRAM") as dramp:
            gi0d = dramp.tile([BT, MSL], F32)
            h1sL = [dramp.tile([HBS, 128, KH * 16], F16, name=f"h1s{j}")
                    for j in range(NB)]
            hstL = [dramp.tile([HBS * 16, 2], F32, name=f"hst{j}")
                    for j in range(NB)]

            # ============ Phase 0: gi0 = x @ Wih0_slice.T + bih0 ============
            with tc.tile_pool(name="p0", bufs=1) as p0, \
                 tc.tile_pool(name="p0o", bufs=3) as p0o, \
                 tc.tile_pool(name="ps0", bufs=2, space="PSUM") as ps0:
                xs = p0.tile([128, KE, BT], F32)
                nc.sync.dma_start(xs[:], xT.rearrange("k p n -> p k n"))
                wih0s = p0.tile([128, KE, MSL], F32)
                nc.sync.dma_start(wih0s[:],
                                  wih0.rearrange("(k p) m -> p k m", p=128))
                bc0s = p0.tile([128, MSL], F32)
                nc.sync.dma_start(bc0s[:], bc0[:])
                for blk in range(BT // 128):
                    ps = ps0.tile([128, MSL], F32, tag="ps")
                    for ke in range(KE):
                        nc.tensor.matmul(ps[:],
                                         xs[:, ke, blk * 128:(blk + 1) * 128],
                                         wih0s[:, ke, :],
                                         start=(ke == 0), stop=(ke == KE - 1))
                    gt = p0o.tile([128, MSL], F32, tag="gt")
                    nc.vector.scalar_tensor_tensor(gt[:], ps[:], 1.0, bc0s[:],
                                                   ALU.mult, ALU.add)
                    nc.sync.dma_start(gi0d[blk * 128:(blk + 1) * 128, :], gt[:])

            # ============ Phase 1: recurrence + interleaved head ============
            with tc.tile_pool(name="rw", bufs=1) as rw, \
                 tc.tile_pool(name="rs", bufs=2) as rs, \
                 tc.tile_pool(name="rt", bufs=2) as rt, \
                 tc.tile_pool(name="rb", bufs=1) as rb, \
                 tc.tile_pool(name="rk", bufs=2) as rk, \
                 tc.tile_pool(name="hw", bufs=1) as hw, \
                 tc.tile_pool(name="hh", bufs=1) as hh, \
                 tc.tile_pool(name="ho", bufs=1) as ho, \
                 tc.tile_pool(name="rps", bufs=1, space="PSUM") as rps, \
                 tc.tile_pool(name="rpt", bufs=1, space="PSUM") as rpt, \
                 tc.tile_pool(name="hp1p", bufs=1, space="PSUM") as hp1p, \
                 tc.tile_pool(name="hp2p", bufs=1, space="PSUM") as hp2p, \
                 tc.tile_pool(name="hpss", bufs=1, space="PSUM") as hpss, \
                 tc.tile_pool(name="rdram", bufs=3, space="DRAM") as rdram:

                # ---- recurrence constants ----
                whh0s = rw.tile([128, KH, MSL], F32)
                nc.sync.dma_start(whh0s[:],
                                  whh0.rearrange("(k p) m -> p k m", p=128))
                wih1s = rw.tile([128, KH, MSL], F32)
                nc.sync.dma_start(wih1s[:],
                                  wih1.rearrange("(k p) m -> p k m", p=128))
                whh1s = rw.tile([128, KH, MSL], F32)
                nc.sync.dma_start(whh1s[:],
                                  whh1.rearrange("(k p) m -> p k m", p=128))
                cin0s = rw.tile([16, MSL], F32)
                nc.sync.dma_start(cin0s[:], cin0[:])
                cinIs = rw.tile([16, MSL], F32)
                nc.sync.dma_start(cinIs[:], cinI[:])
                cinHs = rw.tile([16, MSL], F32)
                nc.sync.dma_start(cinHs[:], cinH[:])
                vw0s = rw.tile([16, MSL], F32)
                nc.sync.dma_start(vw0s[:], vw0b[:])
                vwis = rw.tile([16, MSL], F32)
                nc.sync.dma_start(vwis[:], vwib[:])
                vwhs = rw.tile([16, MSL], F32)
                nc.sync.dma_start(vwhs[:], vwhb[:])
                wT0s = rw.tile([16, 128], F32)
                nc.sync.dma_start(wT0s[:], wT0b[:])
                wT1s = rw.tile([16, 128], F32)
                nc.sync.dma_start(wT1s[:], wT1b[:])
                lnb0s = rw.tile([16, 128], F32)
                nc.sync.dma_start(lnb0s[:], lnbT0[:])
                lnb1s = rw.tile([16, 128], F32)
                nc.sync.dma_start(lnb1s[:], lnbT1[:])
                eyes = rw.tile([16, 16], F32)
                nc.sync.dma_start(eyes[:], eye16[:])
                ktile = rw.tile([16, 2], I32)
                nc.vector.memset(ktile[:], QK)
                one_i = rw.tile([16, 2], I32)
                nc.vector.memset(one_i[:], 1)
                c15 = rw.tile([16, 2], F32)
                nc.vector.memset(c15[:], 1.5)
                neghalf = rw.tile([16, 2], F32)
                nc.vector.memset(neghalf[:], -0.5)
                negone = rw.tile([16, 2], F32)
                nc.vector.memset(negone[:], -1.0)
                epsT = rw.tile([16, 2], F32)
                nc.vector.memset(epsT[:], EPS)
                z128 = rw.tile([16, 128], F32)
                nc.vector.memset(z128[:], 0.0)
                eye1 = rw.tile([1, 1], F32)
                nc.vector.memset(eye1[:], 1.0)

                # ---- head constants ----
                w1s = hw.tile([128, KH, H], F16)
                nc.sync.dma_start(w1s[:],
                                  w1sb.rearrange("(k p) m -> p k m", p=128))
                w1x = hw.tile([2, H], F16)
                nc.sync.dma_start(w1x[:], w1xb[:])
                w2s = hw.tile([128, KH, VC], F16)
                nc.sync.dma_start(w2s[:],
                                  w2sb.rearrange("(k p) v -> p k v", p=128))
                w2x = hw.tile([2, VC], F16)
                nc.sync.dma_start(w2x[:], w2xb[:])
                ones16 = hw.tile([128, 1], F16)
                nc.vector.memset(ones16[:], 1.0)

                # ---- head op-group closures ----
                def mk_load(j):
                    def f(st):
                        hkb = hh.tile([128, KH, HBS, 16], F16, tag="hkb")
                        for c in range(KH):
                            eng = nc.sync if c % 2 == 0 else nc.scalar
                            eng.dma_start(
                                hkb[:, c],
                                h1sL[j][:, :, c * 16:(c + 1) * 16].rearrange(
                                    "t p b -> p t b"))
                        hst = ho.tile([2, 512], F32, tag="hst")
                        nc.gpsimd.dma_start(
                            hst[:], hstL[j].rearrange("n s -> s n"))
                        hstb = hh.tile([2, 512], F16, tag="hstb")
                        nc.vector.tensor_copy(hstb[:], hst[:])
                        st['hkb'] = hkb
                        st['hstb'] = hstb
                        st['ab'] = hh.tile([128, KH, 512], F16, tag="ab", name="ab")
                        st['ps_s'] = hpss.tile([1, 512], F32, tag="ps_s", name="ps_s")
                        st['ps_q'] = hpss.tile([1, 512], F32, tag="ps_q", name="ps_q")
                    return f

                def mk_w1(j, m):
                    def f(st):
                        hkb, hstb, ab = st['hkb'], st['hstb'], st['ab']
                        ps_s, ps_q = st['ps_s'], st['ps_q']
                        ps_a = hp1p.tile([128, 512], F32, tag="ps_a")
                        for kk in range(KH):
                            nc.tensor.matmul(ps_a[:],
                                             w1s[:, kk, m * 128:(m + 1) * 128],
                                             hkb[:, kk],
                                             start=(kk == 0), stop=False)
                        nc.tensor.matmul(ps_a[:], w1x[:, m * 128:(m + 1) * 128],
                                         hstb[:], start=False, stop=True)
                        lt = ho.tile([128, 512], F32, tag="lt")
                        nc.vector.tensor_scalar_mul(lt[:], ps_a[:], NEG_SLOPE)
                        nc.vector.tensor_tensor(ab[:, m], lt[:], ps_a[:],
                                                ALU.max)
                        nc.tensor.matmul(ps_s[:], ones16[:], ab[:, m],
                                         start=(m == 0), stop=(m == KH - 1))
                        sq = ho.tile([128, 512], F16, tag="sq")
                        nc.scalar.activation(sq[:], ab[:, m], AF.Square)
                        nc.tensor.matmul(ps_q[:], ones16[:], sq[:],
                                         start=(m == 0), stop=(m == KH - 1))
                    return f

                def mk_fin(j):
                    def f(st):
                        ps_s, ps_q = st['ps_s'], st['ps_q']
                        m2 = ho.tile([1, 512], F32, tag="m2")
                        nc.vector.tensor_scalar_mul(m2[:], ps_s[:], 1.0 / H)
                        v2 = ho.tile([1, 512], F32, tag="v2")
                        nc.vector.tensor_tensor(v2[:], m2[:], m2[:], ALU.mult)
                        nc.vector.scalar_tensor_tensor(
                            v2[:], ps_q[:], 1.0 / H, v2[:],
                            ALU.mult, ALU.subtract)
                        nc.vector.tensor_scalar_add(v2[:], v2[:], EPS)
                        s2 = ho.tile([1, 512], F32, tag="s2")
                        nc.scalar.activation(s2[:], v2[:], AF.Sqrt)
                        hxa = ho.tile([1, 512], F16, tag="hxa")
                        nc.vector.tensor_scalar_mul(hxa[:], m2[:], -1.0)
                        hxb = ho.tile([1, 512], F16, tag="hxb")
                        nc.vector.tensor_copy(hxb[:], s2[:])
                        hx = hh.tile([2, 512], F16, tag="hx")
                        nc.gpsimd.dma_start(hx[0:1, :], hxa[:])
                        nc.gpsimd.dma_start(hx[1:2, :], hxb[:])
                        rr = m2
                        nc.vector.reciprocal(rr[:], s2[:])
                        r2T = hh.tile([128, 4], F32, tag="r2T")
                        for cc in range(4):
                            ptr = rpt.tile([128, 16], F32, tag="pt")
                            nc.tensor.transpose(
                                ptr[:, 0:1], rr[:, cc * 128:(cc + 1) * 128],
                                eye1[:])
                            nc.scalar.copy(r2T[:, cc:cc + 1], ptr[:, 0:1])
                        st['hx'] = hx
                        st['r2T'] = r2T
                    return f

                def mk_w2(j, cc, vs):
                    def f(st):
                        ab, r2T, hx = st['ab'], st['r2T'], st['hx']
                        pv = hp2p.tile([128, 500], F32, tag="pv")
                        for kk in range(KH):
                            nc.tensor.matmul(
                                pv[:], ab[:, kk, cc * 128:(cc + 1) * 128],
                                w2s[:, kk, vs * 500:(vs + 1) * 500],
                                start=(kk == 0), stop=False)
                        nc.tensor.matmul(pv[:],
                                         hx[:, cc * 128:(cc + 1) * 128],
                                         w2x[:, vs * 500:(vs + 1) * 500],
                                         start=False, stop=True)
                        ot = ho.tile([128, 500], F32, tag="ot")
                        nc.scalar.activation(ot[:], pv[:], AF.Identity,
                                             scale=r2T[:, cc:cc + 1])
                        nc.sync.dma_start(
                            out[j * 512 + cc * 128:j * 512 + (cc + 1) * 128,
                                vs * 500:(vs + 1) * 500], ot[:])
                    return f

                groups = []
                hstate = [dict() for _ in range(NB)]
                for j in range(NB):
                    groups.append((j, mk_load(j)))
                    for m in range(KH):
                        groups.append((j, mk_w1(j, m)))
                    groups.append((j, mk_fin(j)))
                    for cc in range(4):
                        for vs in range(8):
                            groups.append((j, mk_w2(j, cc, vs)))
                gptr = [0]

                def pull(k, budget):
                    while budget > 0 and gptr[0] < len(groups):
                        j, fn = groups[gptr[0]]
                        if k is not None and k < HBS * j + HBS + 2:
                            return
                        fn(hstate[j])
                        gptr[0] += 1
                        budget -= 1

                u0T_prev = z128
                u1T_prev = z128
                u1T_prev2 = z128
                agoA_prev = None
                agoB_prev = None

                for k in range(T + 2):
                    lastA = (k >= T)
                    lastB = (k >= T + 1)

                    agg = rt.tile([16, 2, 2], F32, tag="agg")
                    if lastB:
                        nc.vector.memset(agg[:], 0.0)

                    # ---------- consume AG_A[k-1] ----------
                    if k <= T:
                        u0g = rs.tile([128, KH, 16], F32, tag="u0g")
                        st0in = rt.tile([16, KH, 8], F32, tag="st0in")
                        if k == 0:
                            nc.vector.memset(u0g[:], 0.0)
                            nc.vector.memset(st0in[:], 0.0)
                            nc.vector.memset(st0in[:, :, 0], 64.0)
                            nc.vector.memset(st0in[:, :, 3], 64.0)
                        else:
                            nc.sync.dma_start(
                                u0g[:], agoA_prev[:, 0:2048].rearrange(
                                    "c (p b) -> p c b", p=128))
                            nc.gpsimd.dma_start(
                                st0in[:, :, 0:6],
                                agoA_prev[:, 2048:2144].rearrange(
                                    "c (b s) -> b c s", b=16))
                        nc.vector.bn_aggr(agg[:, 0], st0in[:, :, 0:6])

                    # ---------- consume AG_B[k-1] ----------
                    u1g = rs.tile([128, KH, 16], F32, tag="u1g")
                    st1in = rt.tile([16, KH, 8], F32, tag="st1in")
                    if k <= 1:
                        nc.vector.memset(u1g[:], 0.0)
                        nc.vector.memset(st1in[:], 0.0)
                        nc.vector.memset(st1in[:, :, 0], 64.0)
                        nc.vector.memset(st1in[:, :, 3], 64.0)
                    else:
                        nc.sync.dma_start(
                            u1g[:], agoB_prev[:, 0:2048].rearrange(
                                "c (p b) -> p c b", p=128))
                        nc.gpsimd.dma_start(
                            st1in[:, :, 0:6],
                            agoB_prev[:, 2048:2144].rearrange(
                                "c (b s) -> b c s", b=16))
                    nc.vector.bn_aggr(agg[:, 1], st1in[:, :, 0:6])

                    # ---------- batched stats -> rstd, mrs ----------
                    g = nc.gpsimd
                    ve = rt.tile([16, 2], F32, tag="ve")
                    g.tensor_tensor(ve[:], agg[:, :, 1], epsT[:], ALU.add)
                    jt = rt.tile([16, 2], I32, tag="jt")
                    nc.vector.tensor_tensor(jt[:], ve[:].bitcast(I32),
                                            one_i[:], ALU.logical_shift_right)
                    rstd = rt.tile([16, 2], F32, tag="rstd")
                    g.tensor_tensor(rstd[:].bitcast(I32), ktile[:], jt[:],
                                    ALU.subtract)
                    t1 = rt.tile([16, 2], F32, tag="t1")
                    t2 = rt.tile([16, 2], F32, tag="t2")
                    for _ in range(3):
                        g.tensor_tensor(t1[:], rstd[:], rstd[:], ALU.mult)
                        g.tensor_tensor(t2[:], t1[:], ve[:], ALU.mult)
                        g.tensor_tensor(t2[:], t2[:], neghalf[:], ALU.mult)
                        g.tensor_tensor(t2[:], t2[:], c15[:], ALU.add)
                        g.tensor_tensor(rstd[:], rstd[:], t2[:], ALU.mult)
                    mrs_p = rt.tile([16, 2], F32, tag="mrsp")
                    nc.vector.tensor_tensor(mrs_p[:], agg[:, :, 0], rstd[:],
                                            ALU.mult)
                    mrs_n = rt.tile([16, 2], F32, tag="mrsn")
                    g.tensor_tensor(mrs_n[:], mrs_p[:], negone[:], ALU.mult)
                    rstd0, rstd1 = rstd[:, 0:1], rstd[:, 1:2]
                    mrs_p0, mrs_p1 = mrs_p[:, 0:1], mrs_p[:, 1:2]
                    mrs_n0, mrs_n1 = mrs_n[:, 0:1], mrs_n[:, 1:2]
                    # head stats + u1 store for t=k-2
                    if 2 <= k:
                        hs = rt.tile([16, 2], F32, tag="hs")
                        g.tensor_tensor(hs[:, 0:1], agg[:, 1:2, 0],
                                        negone[:, 0:1], ALU.mult)
                        g.tensor_tensor(hs[:, 1:2], ve[:, 1:2], rstd1,
                                        ALU.mult)
                        jb, tt_ = (k - 2) // HBS, (k - 2) % HBS
                        g.dma_start(hstL[jb][tt_ * 16:(tt_ + 1) * 16, :], hs[:])
                        u1c = rt.tile([128, KH * 16], F16, tag="u1c")
                        nc.scalar.copy(u1c[:],
                                       u1g[:].rearrange("p c b -> p (c b)"))
                        nc.scalar.dma_start(h1sL[jb][tt_], u1c[:])

                    # ---------- gate matmul chains ----------
                    if not lastA:
                        pS0 = rps.tile([16, MSL], F32, tag="pS0")
                        for kk in range(KH):
                            nc.tensor.matmul(pS0[:], u0g[:, kk, :],
                                             whh0s[:, kk, :],
                                             start=(kk == 0), stop=(kk == KH - 1))
                    if not lastB:
                        pS1i = rps.tile([16, MSL], F32, tag="pS1i")
                        for kk in range(KH):
                            nc.tensor.matmul(pS1i[:], u0g[:, kk, :],
                                             wih1s[:, kk, :],
                                             start=(kk == 0), stop=(kk == KH - 1))
                        pS1h = rps.tile([16, MSL], F32, tag="pS1h")
                        for kk in range(KH):
                            nc.tensor.matmul(pS1h[:], u1g[:, kk, :],
                                             whh1s[:, kk, :],
                                             start=(kk == 0), stop=(kk == KH - 1))

                    # ---------- layer0 gates -> h0raw[k] ----------
                    if not lastA:
                        gi0c = rk.tile([16, MSL], F32, tag="gi0c")
                        nc.scalar.dma_start(
                            gi0c[:], gi0d[k * 16:(k + 1) * 16, :])
                        corr0 = rb.tile([16, MSL], F32, tag="corr0")
                        nc.vector.scalar_tensor_tensor(
                            corr0[:], vw0s[:], mrs_p0, cin0s[:],
                            ALU.mult, ALU.subtract)
                        pre0 = rb.tile([16, MSL], F32, tag="pre0")
                        nc.vector.scalar_tensor_tensor(
                            pre0[:], pS0[:], rstd0, corr0[:],
                            ALU.mult, ALU.subtract)
                        rz0 = rt.tile([16, 256], F32, tag="rz0")
                        nc.gpsimd.tensor_tensor(rz0[:], pre0[:, 0:256],
                                                gi0c[:, 0:256], ALU.add)
                        sg0 = rt.tile([16, 256], F32, tag="sg0")
                        nc.scalar.activation(sg0[:], rz0[:], AF.Sigmoid)
                        n0a = rt.tile([16, 128], F32, tag="n0a")
                        nc.vector.tensor_tensor(n0a[:], sg0[:, 0:128],
                                                pre0[:, 256:384], ALU.mult)
                        nc.gpsimd.tensor_tensor(n0a[:], n0a[:],
                                                gi0c[:, 256:384], ALU.add)
                        n0 = rt.tile([16, 128], F32, tag="n0")
                        nc.scalar.activation(n0[:], n0a[:], AF.Tanh)
                        q0 = rt.tile([16, 128], F32, tag="q0")
                        nc.vector.scalar_tensor_tensor(
                            q0[:], wT0s[:], mrs_n0, lnb0s[:],
                            ALU.mult, ALU.add)
                        hp0 = rt.tile([16, 128], F32, tag="hp0")
                        nc.vector.scalar_tensor_tensor(
                            hp0[:], u0T_prev[:], rstd0, q0[:],
                            ALU.mult, ALU.add)
                        d0 = rt.tile([16, 128], F32, tag="d0")
                        nc.gpsimd.tensor_tensor(d0[:], hp0[:], n0[:],
                                                ALU.subtract)
                        nc.vector.tensor_tensor(d0[:], d0[:], sg0[:, 128:256],
                                                ALU.mult)
                        h0n = rt.tile([16, 128], F32, tag="h0n")
                        nc.gpsimd.tensor_tensor(h0n[:], d0[:], n0[:], ALU.add)
                        u0T = rk.tile([16, 128], F32, tag="u0T")
                        nc.vector.tensor_tensor(u0T[:], h0n[:], wT0s[:],
                                                ALU.mult)
                        st0 = rt.tile([16, 6], F32, tag="st0")
                        nc.vector.bn_stats(st0[:], h0n[:])
                        pt0 = rpt.tile([128, 16], F32, tag="pt")
                        nc.tensor.transpose(pt0[:], u0T[:], eyes[:])
                        u0s = rt.tile([128, 16], F32, tag="u0s")
                        nc.scalar.copy(u0s[:], pt0[:])
                        aginA = rdram.tile([PAY], F32, tag="aginA")
                        nc.sync.dma_start(
                            aginA[0:2048].rearrange("(p b) -> p b", p=128),
                            u0s[:])
                        nc.sync.dma_start(
                            aginA[2048:2144].rearrange("(b s) -> b s", b=16),
                            st0[:])
                        agoA = rdram.tile([n_cores, PAY], F32,
                                          tag="agoA", addr_space="Shared")
                        nc.gpsimd.collective_compute(
                            "AllGather", ALU.bypass, replica_groups=rg,
                            ins=[aginA.opt()], outs=[agoA.opt()])
                        u0T_prev = u0T
                        agoA_prev = agoA

                    # ---------- layer1 gates -> h1raw[k-1] ----------
                    if not lastB:
                        cA = rb.tile([16, MSL], F32, tag="cA")
                        nc.vector.scalar_tensor_tensor(
                            cA[:], vwis[:], mrs_p0, cinIs[:],
                            ALU.mult, ALU.subtract)
                        gA = rb.tile([16, MSL], F32, tag="gA")
                        nc.vector.scalar_tensor_tensor(
                            gA[:], pS1i[:], rstd0, cA[:],
                            ALU.mult, ALU.subtract)
                        cB = rb.tile([16, MSL], F32, tag="cB")
                        nc.vector.scalar_tensor_tensor(
                            cB[:], vwhs[:], mrs_p1, cinHs[:],
                            ALU.mult, ALU.subtract)
                        gB = rb.tile([16, MSL], F32, tag="gB")
                        nc.vector.scalar_tensor_tensor(
                            gB[:], pS1h[:], rstd1, cB[:],
                            ALU.mult, ALU.subtract)
                        rz1 = rt.tile([16, 256], F32, tag="rz1")
                        nc.gpsimd.tensor_tensor(rz1[:], gA[:, 0:256],
                                                gB[:, 0:256], ALU.add)
                        sg1 = rt.tile([16, 256], F32, tag="sg1")
                        nc.scalar.activation(sg1[:], rz1[:], AF.Sigmoid)
                        n1a = rt.tile([16, 128], F32, tag="n1a")
                        nc.vector.tensor_tensor(n1a[:], sg1[:, 0:128],
                                                gB[:, 256:384], ALU.mult)
                        nc.gpsimd.tensor_tensor(n1a[:], n1a[:],
                                                gA[:, 256:384], ALU.add)
                        n1 = rt.tile([16, 128], F32, tag="n1")
                        nc.scalar.activation(n1[:], n1a[:], AF.Tanh)
                        q1 = rt.tile([16, 128], F32, tag="q1")
                        nc.vector.scalar_tensor_tensor(
                            q1[:], wT1s[:], mrs_n1, lnb1s[:],
                            ALU.mult, ALU.add)
                        hp1 = rt.tile([16, 128], F32, tag="hp1")
                        up1 = u1T_prev2 if k == 1 else u1T_prev
                        nc.vector.scalar_tensor_tensor(
                            hp1[:], up1[:], rstd1, q1[:],
                            ALU.mult, ALU.add)
                        d1 = rt.tile([16, 128], F32, tag="d1")
                        nc.gpsimd.tensor_tensor(d1[:], hp1[:], n1[:],
                                                ALU.subtract)
                        nc.vector.tensor_tensor(d1[:], d1[:], sg1[:, 128:256],
                                                ALU.mult)
                        h1n = rt.tile([16, 128], F32, tag="h1n")
                        nc.gpsimd.tensor_tensor(h1n[:], d1[:], n1[:], ALU.add)
                        u1T = rk.tile([16, 128], F32, tag="u1T")
                        nc.vector.tensor_tensor(u1T[:], h1n[:], wT1s[:],
                                                ALU.mult)
                        st1 = rt.tile([16, 6], F32, tag="st1")
                        nc.vector.bn_stats(st1[:], h1n[:])
                        pt1 = rpt.tile([128, 16], F32, tag="pt")
                        nc.tensor.transpose(pt1[:], u1T[:], eyes[:])
                        u1s = rt.tile([128, 16], F32, tag="u1s")
                        nc.scalar.copy(u1s[:], pt1[:])
                        aginB = rdram.tile([PAY], F32, tag="aginB")
                        nc.sync.dma_start(
                            aginB[0:2048].rearrange("(p b) -> p b", p=128),
                            u1s[:])
                        nc.sync.dma_start(
                            aginB[2048:2144].rearrange("(b s) -> b s", b=16),
                            st1[:])
                        agoB = rdram.tile([n_cores, PAY], F32,
                                          tag="agoB", addr_space="Shared")
                        nc.gpsimd.collective_compute(
                            "AllGather", ALU.bypass, replica_groups=rg,
                            ins=[aginB.opt()], outs=[agoB.opt()])
                        u1T_prev = u1T
                        agoB_prev = agoB

                    # ---------- pull head work ----------
                    if INTERLEAVE_HEAD:
                        pull(k, 2)

                # drain remaining head groups
                pull(None, 10 ** 9)
    return nc


# ===================== host-side prep / post =====================

def _np(x):
    return np.asarray(x)


def prep_in_maps(inputs, n_cores=NCORES):
    F16N = np.float16
    ids = _np(inputs['input']).astype(np.int64)[:, :T]
    embd = _np(inputs['embd']).astype(np.float32)
    x = embd[ids]
    xT = np.ascontiguousarray(x.transpose(2, 1, 0).reshape(E, BT))
    xT = xT.reshape(KE, 128, BT)

    def gate_slice(W, c):
        cols = []
        for g in range(3):
            cols.append(W[g * H + c * 128:(g * H + (c + 1) * 128), :])
        return np.ascontiguousarray(np.concatenate(cols, axis=0).T)

    def vec_slice(v, c):
        return np.concatenate([v[g * H + c * 128:g * H + (c + 1) * 128]
                               for g in range(3)])

    Wih0 = _np(inputs['Wih0']).astype(np.float32)
    Whh0 = _np(inputs['Whh0']).astype(np.float32)
    Wih1 = _np(inputs['Wih1']).astype(np.float32)
    Whh1 = _np(inputs['Whh1']).astype(np.float32)
    bih0 = _np(inputs['bih0']).astype(np.float32)
    bhh0 = _np(inputs['bhh0']).astype(np.float32)
    bih1 = _np(inputs['bih1']).astype(np.float32)
    bhh1 = _np(inputs['bhh1']).astype(np.float32)
    lnw0 = _np(inputs['ln0_w']).astype(np.float32)
    lnb0 = _np(inputs['ln0_b']).astype(np.float32)
    lnw1 = _np(inputs['ln1_w']).astype(np.float32)
    lnb1 = _np(inputs['ln1_b']).astype(np.float32)
    ln2w = _np(inputs['ln2_w']).astype(np.float32)
    ln2b = _np(inputs['ln2_b']).astype(np.float32)
    W1 = _np(inputs['W1']).astype(np.float32)
    b1 = _np(inputs['b1']).astype(np.float32)
    W2 = _np(inputs['W2']).astype(np.float32)

    w1T = np.ascontiguousarray(W1.T)
    vw1 = lnw1 @ w1T
    vb1 = lnb1 @ w1T + b1
    w1sb = w1T.astype(F16N)
    w1xb = np.stack([vw1, vb1]).astype(F16N)
    eye = np.eye(16, dtype=np.float32)

    in_maps = []
    for c in range(n_cores):
        whh0c = gate_slice(Whh0, c)
        wih1c = gate_slice(Wih1, c)
        whh1c = gate_slice(Whh1, c)
        wih0c = gate_slice(Wih0, c)
        vw0 = lnw0 @ whh0c
        vb0 = lnb0 @ whh0c
        vwi = lnw0 @ wih1c
        vbi = lnb0 @ wih1c
        vwh = lnw1 @ whh1c
        vbh = lnb1 @ whh1c
        bc0c = np.tile(vec_slice(bih0, c)[None, :], (128, 1))
        cin0c = np.tile((vec_slice(bhh0, c) + vb0)[None, :], (16, 1))
        cinIc = np.tile((vec_slice(bih1, c) + vbi)[None, :], (16, 1))
        cinHc = np.tile((vec_slice(bhh1, c) + vbh)[None, :], (16, 1))
        W2c = W2[c * VC:(c + 1) * VC, :]
        w2eff = (W2c * ln2w[None, :]).T
        vw2 = W2c @ ln2w
        vb2 = W2c @ ln2b
        in_maps.append({
            'xT': xT, 'wih0': wih0c, 'whh0': whh0c,
            'wih1': wih1c, 'whh1': whh1c,
            'bc0': np.ascontiguousarray(bc0c, np.float32),
            'cin0': np.ascontiguousarray(cin0c, np.float32),
            'cinI': np.ascontiguousarray(cinIc, np.float32),
            'cinH': np.ascontiguousarray(cinHc, np.float32),
            'vw0b': np.ascontiguousarray(np.tile(vw0[None, :], (16, 1)),
                                         np.float32),
            'vwib': np.ascontiguousarray(np.tile(vwi[None, :], (16, 1)),
                                         np.float32),
            'vwhb': np.ascontiguousarray(np.tile(vwh[None, :], (16, 1)),
                                         np.float32),
            'wT0b': np.ascontiguousarray(
                np.tile(lnw0[c * 128:(c + 1) * 128][None, :], (16, 1)),
                np.float32),
            'wT1b': np.ascontiguousarray(
                np.tile(lnw1[c * 128:(c + 1) * 128][None, :], (16, 1)),
                np.float32),
            'lnbT0': np.ascontiguousarray(
                np.tile(lnb0[c * 128:(c + 1) * 128][None, :], (16, 1)),
                np.float32),
            'lnbT1': np.ascontiguousarray(
                np.tile(lnb1[c * 128:(c + 1) * 128][None, :], (16, 1)),
                np.float32),
            'eye16': eye,
            'w1sb': np.ascontiguousarray(w1sb),
            'w1xb': np.ascontiguousarray(w1xb),
            'w2sb': np.ascontiguousarray(w2eff.astype(F16N)),
            'w2xb': np.ascontiguousarray(np.stack([vw2, vb2]).astype(F16N)),
        })
    return in_maps


def postprocess(results, inputs):
    b2 = _np(inputs['b2']).astype(np.float32)
    full = np.concatenate([r['out'] for r in results], axis=1)
    full = full.reshape(T, B, V).transpose(1, 0, 2)
    return full + b2


# ===================== NEFF disk cache =====================

def _install_neff_cache():
    import hashlib, os, shutil
    import concourse.bass2jax as b2j
    from concourse.bass_utils import compile_bir_kernel as _real
    if getattr(b2j, "_ant_neff_cache_installed", False):
        return
    cache_dir = os.path.expanduser("~/.cache/bass_neff_cache")
    os.makedirs(cache_dir, exist_ok=True)

    def cached(bir_json, tmpdir, neff_name="file.neff"):
        key = hashlib.sha256(bir_json).hexdigest()
        p = os.path.join(cache_dir, key + ".neff")
        out = os.path.join(tmpdir, neff_name)
        if os.path.exists(p):
            shutil.copyfile(p, out)
            return out
        r = _real(bir_json, tmpdir, neff_name)
        try:
            shutil.copyfile(r, p)
        except OSError:
            pass
        return r

    b2j.compile_bir_kernel = cached
    b2j._ant_neff_cache_installed = True


# ===================== NTFF profile shim (for traced runs) ==================

def _install_axon_prof():
    import types, ctypes, contextlib
    try:
        from antenv import axon_hooks  # noqa: F401
        return
    except ImportError:
        pass
    so_path = "/opt/axon/libaxon_pjrt.so"
    try:
        lib = ctypes.CDLL(so_path)
    except OSError:
        return
    hook = None
    if hasattr(lib, "axon_start_nrt_profile"):
        lib.axon_start_nrt_profile.argtypes = [
            ctypes.POINTER(ctypes.c_int64), ctypes.c_size_t]
        lib.axon_start_nrt_profile.restype = ctypes.c_int64
        lib.axon_stop_nrt_profile.argtypes = [ctypes.c_char_p]
        lib.axon_stop_nrt_profile.restype = ctypes.c_int64

        @contextlib.contextmanager
        def hook(output_dir, device_ids):
            import jax
            jax.devices()
            if device_ids:
                ids = (ctypes.c_int64 * len(device_ids))(*device_ids)
                rc = lib.axon_start_nrt_profile(ids, len(device_ids))
            else:
                rc = lib.axon_start_nrt_profile(None, 0)
            if rc != 0:
                raise RuntimeError(f"axon_start_nrt_profile rc={rc}")
            try:
                yield
            finally:
                lib.axon_stop_nrt_profile(str(output_dir).encode())

    mod = types.ModuleType("antenv.axon_hooks")
    _h = [hook]
    mod.set_axon_ntff_profile_hook = lambda h: _h.__setitem__(0, h)
    mod.get_axon_ntff_profile_hook = lambda: _h[0]
    _sys.modules["antenv.axon_hooks"] = mod
    import antenv
    antenv.axon_hooks = mod


# ===================== entry point =====================

_NC = None


def _get_nc():
    global _NC
    if _NC is None:
        _install_neff_cache()
        nc = build_nc()
        nc.compile()
        _NC = nc
    return _NC


def kernel(**inputs):
    from concourse import bass_utils
    nc = _get_nc()
    in_maps = prep_in_maps(inputs)
    res = bass_utils.run_bass_kernel_spmd(
        nc, in_maps, core_ids=list(range(NCORES)))
    return postprocess(res.results, inputs)


def kernel_traced(**inputs):
    """Like kernel() but also returns neuron-profile exec_time_ns."""
    from concourse import bass_utils
    _install_axon_prof()
    nc = _get_nc()
    in_maps = prep_in_maps(inputs)
    res = bass_utils.run_bass_kernel_spmd(
        nc, in_maps, core_ids=list(range(NCORES)), trace=True)
    return postprocess(res.results, inputs), res.exec_time_ns
